# revision 8
# baseline (speedup 1.0000x reference)
"""Single-head causal attention (B=16, S=2048, D=1024, Dh=128) on 8 TRN2 cores.

Sharding: data-parallel over batch — each core computes 2 full batches.

Precision scheme (PE fp32 matmul is 4 cycles/row on TRN2; fp16 is 1): all
matmuls run in fp16 with hi/lo pair splitting where precision matters.
x = xh + xl (fp16 pair, ~22 mantissa bits), W likewise. Projections use 3
passes (xh*Wh + xh*Wl + xl*Wh, fp32 PSUM accumulation), V uses 2 passes
(x exact * Wvh). q'/k are re-split into fp16 pairs for the scores matmul
(3 passes). Softmax is exact fp32 (PSUM scores, DVE row-max, ACT exp with
fused row-sum). attn weights fp16; attn^T and V^T via single-instruction
3D xbar DMA transposes; AV accumulates into the scores PSUM region; the
1/rowsum is folded into the ACT eviction. Measured end-to-end error vs the
fp32 reference: ~5e-4 of output scale.
"""

import numpy as np

import concourse.bass as bass
import concourse.mybir as mybir
import concourse.tile as tile
from concourse import bacc

F32 = mybir.dt.float32
F16 = mybir.dt.float16
NEG_BIG = -1e30

B_FULL = 16
S_FULL = 2048
D_FULL = 1024
DH = 128
N_CORES = 8


def attention_body(tc, x, wq, wk, wv, bq, bk, bv, out, *, S, D, scale):
    nc = tc.nc
    NT = S // 128   # seq tiles
    KC = D // 128   # contraction chunks
    NB = x.shape[0]  # batches per core
    NCH = S // 512  # 512-wide output chunks

    with tc.tile_pool(name="const", bufs=1) as const, \
         tc.tile_pool(name="xa", bufs=4) as xa, \
         tc.tile_pool(name="xb", bufs=6) as xb, \
         tc.tile_pool(name="xt", bufs=1) as xtp, \
         tc.tile_pool(name="qk", bufs=1) as qkp, \
         tc.tile_pool(name="tq", bufs=2) as tqp, \
         tc.tile_pool(name="vv", bufs=2) as vvp, \
         tc.tile_pool(name="pp", bufs=2) as ppp, \
         tc.tile_pool(name="pt", bufs=2) as ptp, \
         tc.tile_pool(name="oo", bufs=3) as oop, \
         tc.tile_pool(name="stats", bufs=12) as stp:

        # --- constants ---
        cmask = const.tile([128, 128], F32)
        nc.gpsimd.memset(cmask, 0.0)
        # keep 0 where q >= k (partition - free >= 0), else NEG_BIG
        nc.gpsimd.affine_select(
            out=cmask, in_=cmask, compare_op=mybir.AluOpType.is_ge,
            fill=NEG_BIG, base=0, pattern=[[-1, 128]], channel_multiplier=1,
        )

        # weights as fp16 hi/lo pairs (V: hi only)
        w_pairs = []
        for wi, w in enumerate((wq, wk, wv)):
            stg = const.tile([128, KC, DH], F32, tag=f"w_stage{wi}")
            nc.sync.dma_start(out=stg, in_=w.rearrange("(c p) h -> p c h", p=128))
            wh = const.tile([128, KC, DH], F16, tag=f"w_hi{wi}")
            nc.vector.tensor_copy(wh, stg)
            if wi < 2:
                wl = const.tile([128, KC, DH], F16, tag=f"w_lo{wi}")
                nc.vector.tensor_sub(wl, stg, wh)
            else:
                wl = None
            w_pairs.append((wh, wl))
        b_sb = []
        for bi, bv_ap in enumerate((bq, bk, bv)):
            t = const.tile([128, 1], F32, tag=f"b_sb{bi}")
            nc.sync.dma_start(out=t, in_=bv_ap)
            b_sb.append(t)
        bq_scaled = const.tile([128, 1], F32)
        nc.vector.tensor_scalar_mul(bq_scaled, b_sb[0], float(scale))

        for b in range(NB):
            xht = xtp.tile([128, KC, S], F16, tag="xht")
            xlt = xtp.tile([128, KC, S], F16, tag="xlt")
            qh = qkp.tile([128, S], F16, tag="qh")
            ql = qkp.tile([128, S], F16, tag="ql")
            kh = qkp.tile([128, S], F16, tag="kh")
            kl = qkp.tile([128, S], F16, tag="kl")
            vt16 = vvp.tile([128, S], F16, tag="vt16")
            vnat = vvp.tile([128, NT, DH], F16, tag="vnat")

            # ---------- phase A: load, fp16-split, transpose ----------
            for i in range(NT):
                sl = slice(i * 128, (i + 1) * 128)
                xn = xa.tile([128, D], F32, tag="xn")
                nc.sync.dma_start(out=xn, in_=x[b, sl, :])
                xhn = xb.tile([128, D], F16, tag="xhn")
                nc.scalar.copy(xhn, xn)
                xln = xb.tile([128, D], F16, tag="xln")
                nc.vector.tensor_sub(xln, xn, xhn)
                nc.sync.dma_start(out=xht[:, :, sl], in_=xhn, transpose=True)
                nc.sync.dma_start(out=xlt[:, :, sl], in_=xln, transpose=True)

            # ---------- phase B: projections ----------
            with tc.tile_pool(name="mmps", bufs=4, space="PSUM") as mmps:
                for (wt, bias_ap, sc_, hi, lo) in (
                    (w_pairs[0], bq_scaled, float(scale), qh, ql),
                    (w_pairs[1], b_sb[1], 1.0, kh, kl),
                ):
                    wh_t, wl_t = wt
                    passes = ((wh_t, xht), (wl_t, xht), (wh_t, xlt))
                    for n in range(NCH):
                        nsl = slice(n * 512, (n + 1) * 512)
                        ps = mmps.tile([128, 512], F32, tag="ps")
                        for pi, (wtile, xtile) in enumerate(passes):
                            for c in range(KC):
                                nc.tensor.matmul(
                                    ps, lhsT=wtile[:, c, :],
                                    rhs=xtile[:, c, nsl],
                                    start=(pi == 0 and c == 0),
                                    stop=(pi == 2 and c == KC - 1),
                                )
                        tmp = tqp.tile([128, 512], F32, tag="tmp")
                        nc.scalar.activation(
                            tmp, ps, mybir.ActivationFunctionType.Identity,
                            bias=bias_ap, scale=sc_,
                        )
                        nc.vector.tensor_copy(hi[:, nsl], tmp)
                        nc.vector.tensor_sub(lo[:, nsl], tmp, hi[:, nsl])
                # V: 2 passes (x exact, W hi only), straight to fp16
                wvh = w_pairs[2][0]
                for n in range(NCH):
                    nsl = slice(n * 512, (n + 1) * 512)
                    ps = mmps.tile([128, 512], F32, tag="ps")
                    for pi, xtile in enumerate((xht, xlt)):
                        for c in range(KC):
                            nc.tensor.matmul(
                                ps, lhsT=wvh[:, c, :], rhs=xtile[:, c, nsl],
                                start=(pi == 0 and c == 0),
                                stop=(pi == 1 and c == KC - 1),
                            )
                    nc.scalar.activation(
                        vt16[:, nsl], ps, mybir.ActivationFunctionType.Identity,
                        bias=b_sb[2], scale=1.0,
                    )
                nc.sync.dma_start(out=vnat, in_=vt16, transpose=True)

            # ---------- phase C: attention ----------
            with tc.tile_pool(name="scps", bufs=2, space="PSUM") as scps:
                for i in range(NT):
                    W = (i + 1) * 128
                    isl = slice(i * 128, (i + 1) * 128)
                    sc = scps.tile([128, S], F32, tag="sc")
                    nch = (W + 511) // 512
                    for n in range(nch):
                        wn = min(512, W - n * 512)
                        nsl = slice(n * 512, n * 512 + wn)
                        for pi, (a_, b_) in enumerate(
                            ((qh, kh), (qh, kl), (ql, kh))
                        ):
                            nc.tensor.matmul(
                                sc[:, nsl], lhsT=a_[:, isl], rhs=b_[:, nsl],
                                start=(pi == 0), stop=(pi == 2),
                            )
                    nc.vector.tensor_add(sc[:, i * 128:W], sc[:, i * 128:W], cmask)
                    negm = stp.tile([128, 1], F32, tag="negm")
                    nc.vector.tensor_reduce(
                        negm, sc[:, :W], axis=mybir.AxisListType.X,
                        op=mybir.AluOpType.max, negate=True,
                    )
                    p = ppp.tile([128, S], F16, tag="p")
                    l = stp.tile([128, 1], F32, tag="l")
                    nc.scalar.activation(
                        p[:, :W], sc[:, :W], mybir.ActivationFunctionType.Exp,
                        bias=negm, scale=1.0, accum_out=l,
                    )
                    r = stp.tile([128, 1], F32, tag="r")
                    nc.vector.reciprocal(r, l)
                    ptile = ptp.tile([128, NT, DH], F16, tag="ptile")
                    nc.sync.dma_start(
                        out=ptile[:, :i + 1, :], in_=p[:, :W], transpose=True,
                    )
                    for j in range(i + 1):
                        nc.tensor.matmul(
                            sc[:, 0:DH], lhsT=ptile[:, j, :], rhs=vnat[:, j, :],
                            start=(j == 0), stop=(j == i),
                        )
                    o = oop.tile([128, DH], F32, tag="o")
                    nc.scalar.mul(o, sc[:, 0:DH], r)
                    nc.sync.dma_start(out=out[b, isl, :], in_=o)


def build_attention_nc(nb=2, S=S_FULL, D=D_FULL):
    # Bacc (not raw Bass): its compile() pass legalizes sync for this
    # toolchain (≤1 wait per instruction, waits moved to ldweights/events).
    nc = bacc.Bacc(trn_type="TRN2")
    x_h = nc.dram_tensor("x", [nb, S, D], F32, kind="ExternalInput")
    wq_h = nc.dram_tensor("Wq", [D, DH], F32, kind="ExternalInput")
    wk_h = nc.dram_tensor("Wk", [D, DH], F32, kind="ExternalInput")
    wv_h = nc.dram_tensor("Wv", [D, DH], F32, kind="ExternalInput")
    bq_h = nc.dram_tensor("bq", [DH, 1], F32, kind="ExternalInput")
    bk_h = nc.dram_tensor("bk", [DH, 1], F32, kind="ExternalInput")
    bv_h = nc.dram_tensor("bv", [DH, 1], F32, kind="ExternalInput")
    out_h = nc.dram_tensor("out", [nb, S, DH], F32, kind="ExternalOutput")
    with tile.TileContext(nc) as tc:
        attention_body(
            tc, x_h.ap(), wq_h.ap(), wk_h.ap(), wv_h.ap(),
            bq_h.ap(), bk_h.ap(), bv_h.ap(), out_h.ap(),
            S=S, D=D, scale=float(D) ** 0.5,
        )
    nc.compile()
    return nc


_NC_CACHE = {}


def _get_nc():
    if "nc" not in _NC_CACHE:
        _NC_CACHE["nc"] = build_attention_nc()
    return _NC_CACHE["nc"]


def make_in_maps(x, Wq, bq, Wk, bk, Wv, bv):
    x = np.ascontiguousarray(np.asarray(x, dtype=np.float32))
    args = {
        "Wq": np.ascontiguousarray(np.asarray(Wq, np.float32)),
        "Wk": np.ascontiguousarray(np.asarray(Wk, np.float32)),
        "Wv": np.ascontiguousarray(np.asarray(Wv, np.float32)),
        "bq": np.ascontiguousarray(np.asarray(bq, np.float32).reshape(DH, 1)),
        "bk": np.ascontiguousarray(np.asarray(bk, np.float32).reshape(DH, 1)),
        "bv": np.ascontiguousarray(np.asarray(bv, np.float32).reshape(DH, 1)),
    }
    nb = x.shape[0] // N_CORES
    return [
        {"x": x[c * nb:(c + 1) * nb], **args} for c in range(N_CORES)
    ]


def kernel(x, Wq, bq, Wk, bk, Wv, bv):
    from concourse.bass_utils import run_bass_kernel_spmd

    nc = _get_nc()
    in_maps = make_in_maps(x, Wq, bq, Wk, bk, Wv, bv)
    res = run_bass_kernel_spmd(nc, in_maps, core_ids=list(range(N_CORES)))
    return np.concatenate([r["out"] for r in res.results], axis=0)
